# revision 31
# baseline (speedup 1.0000x reference)
"""Trainium2 Bass kernel for nn_MultiHeadCrossAttention (B=4, T=1024, E=1024, H=16).

Sharding: 8 fully independent shards, zero cross-core communication:
(output stream s, batch b) for s in {1,2}, b in 0..3. Stream-1 output
xo@Wout1 needs K,V from x and Q from y; stream-2 the reverse.

Per-core kernel (activations transposed, feature-on-partition):
  Preamble: V = A^T.T @ Wv^T (natural, with ones column per head for the
  rowsum trick); Q^T/K^T chunk 0 only (lookahead-1).
  Flat 128-slot loop over (head-pair m, ic, jc); per slot:
    S^T pair via two concurrent K=64 matmuls (tile_position row split),
    pre-issued one slot ahead (including across pair boundaries)
    P = exp(S/8) on ACT into SBUF fp16 (A|B merged per slot)
    O'^T accumulation (M=65 incl. ones row -> rowsum) lagged one slot
    Q^T/K^T projection chunk m+1 interleaved: Q-half slots 0-5 (3/3/3/3/2/2),
    K-half slots 6-13 (2/slot); CASTs at s5/s13 to keep DVE clear of the
    s8/s0 psum evacuations
    normalization of pair m-1 lagged: reciprocal_approx_fast + gpsimd
    partition_broadcast + fp16 multiply into ot
  Tail: Z^T = Wout^T.T @ O^T accumulated over head pairs, fp16 out DMA.
"""

import os
import sys

sys.path.insert(0, "/opt/trn_rl_repo")

import numpy as np
import ml_dtypes
from contextlib import ExitStack

import concourse.bass as bass
import concourse.mybir as mybir
import concourse.tile as tile
from concourse import bacc
from concourse import bass_utils

B, T, E, H = 4, 1024, 1024, 16
D = E // H            # 64
NC = E // 128         # 8 chunks of 128
N_CORES = 8

F32 = mybir.dt.float32
F16 = mybir.dt.float16

_NC_CACHE = {}
LAST_RESULTS = {}


def _proj_schedule():
    """Per (m, s) list of proj ops at column-half (icc) granularity.

    Pair m (m<=6) computes chunk ch=m+1. Quarters of 8 matmuls each, 2 per
    slot, with the 512-wide psum->SBUF cast right after each quarter:
      Q icc0: s0-3  (cast s3)   Q icc1: s4-7   (cast s7)
      K icc0: s8-11 (cast s11)  K icc1: s12-15 (cast s15)
    Each quarter accumulates in its own 1-bank [128,512] pj tile, so every
    pj reuse has >=4 slots of slack behind its cast (no in-order PE stalls).
    Returns dict (m, s) -> list of ('mm', which, icc, e) / ('cast', which, icc).
    """
    sched = {}
    quarters = [('q', 0, 0), ('q', 1, 4), ('k', 0, 8), ('k', 1, 12)]
    for m in range(NC - 1):
        for which, icc, s0 in quarters:
            for k in range(4):
                ops = [('mm', which, icc, 2 * k), ('mm', which, icc, 2 * k + 1)]
                # q-icc0 / k-icc0 cast immediately; q-icc1 / k-icc1 casts are
                # DEFERRED by the body (after the s8 / next-s0 evacuations) so
                # the DVE runs the psum evacuations first and the O matmuls
                # never head-block the in-order PE on the ops-psum reuse.
                if k == 3 and icc == 0:
                    ops.append(('cast', which, icc))
                sched[(m, s0 + k)] = ops
    return sched


_SCHED = _proj_schedule()


def _build():
    nc = bacc.Bacc("TRN2", target_bir_lowering=False, debug=False,
                   enable_asserts=False, num_devices=N_CORES)
    a_t = nc.dram_tensor("a_t", (E, T), F16, kind="ExternalInput").ap()
    b_t = nc.dram_tensor("b_t", (E, T), F16, kind="ExternalInput").ap()
    wq_t = nc.dram_tensor("wq_t", (E, E), F16, kind="ExternalInput").ap()
    wk_t = nc.dram_tensor("wk_t", (E, E), F16, kind="ExternalInput").ap()
    wv_t = nc.dram_tensor("wv_t", (E, E), F16, kind="ExternalInput").ap()
    wout_t = nc.dram_tensor("wout_t", (E, E), F16, kind="ExternalInput").ap()
    z_t = nc.dram_tensor("z_t", (E, T), F16, kind="ExternalOutput").ap()

    EXP = mybir.ActivationFunctionType.Exp

    with tile.TileContext(nc) as tc, ExitStack() as ctx:
        persist = ctx.enter_context(tc.tile_pool(name="persist", bufs=1))
        qt = persist.tile([128, NC, T], F16, tag="qt")
        kt = persist.tile([128, NC, T], F16, tag="kt")
        v = persist.tile([128, NC, H * (D + 1)], F16, tag="v")
        ot = persist.tile([128, NC, T], F16, tag="ot")
        wo_sb = persist.tile([128, NC, E], F16, tag="wo", name="wo_sb")

        for mch in range(NC):
            nc.vector.memset(
                v[:, mch, :].rearrange("p (h x) -> p h x", x=D + 1)[:, :, D:D + 1],
                1.0)

        acts = ctx.enter_context(tc.tile_pool(name="acts", bufs=1))
        at_sb = acts.tile([128, NC, T], F16, tag="at")
        bt_sb = acts.tile([128, NC, T], F16, tag="bt")
        wq_sb = acts.tile([128, NC, E], F16, tag="wq")
        wk_sb = acts.tile([128, NC, E], F16, tag="wk")

        # PE warmup: the first ~12us are DMA-prefix dead time with the PE
        # idle, so the HAM clock-gate would hold the array at 1.2 GHz when
        # the V projection starts. ~28 discarded matmuls on a zeroed tile
        # keep the PE busy through the dead window and release the throttle.
        with tc.tile_pool(name="wrm", bufs=1) as wrm, \
             tc.tile_pool(name="wmp", bufs=1, space="PSUM") as wmp:
            warm = wrm.tile([128, 512], F16, tag="warm")
            nc.vector.memset(warm[:], 0.0)
            wps = wmp.tile([128, 512], F32, tag="wp")
            for i in range(28):
                nc.tensor.matmul(wps[:], warm[:, 0:128], warm[:],
                                 start=(i == 0), stop=(i == 27))

        # ---------------- Preamble: V proj + QK chunk 0 ----------------
        with tc.tile_pool(name="wvp", bufs=1) as wvp, \
             tc.tile_pool(name="pps", bufs=4, space="PSUM") as pps:
            wv_sb = wvp.tile([128, NC, E], F16, tag="wv")
            # DMA priority: (at, wv) pairs first so V proj starts ASAP;
            # then wk (K chunk 0 unblocks), then (bt, wq), then wout.
            for e in range(NC):
                nc.sync.dma_start(at_sb[:, e, :], a_t[e * 128:(e + 1) * 128, :])
                nc.sync.dma_start(wv_sb[:, e, :], wv_t[e * 128:(e + 1) * 128, :])
            for e in range(NC):
                nc.sync.dma_start(wk_sb[:, e, :], wk_t[e * 128:(e + 1) * 128, :])
            for e in range(NC):
                nc.sync.dma_start(bt_sb[:, e, :], b_t[e * 128:(e + 1) * 128, :])
                nc.sync.dma_start(wq_sb[:, e, :], wq_t[e * 128:(e + 1) * 128, :])
            for e in range(NC):
                nc.sync.dma_start(wo_sb[:, e, :], wout_t[e * 128:(e + 1) * 128, :])

            # V natural: out[j-chunk m][h*d] = sum_e at[e, j].T @ wv[e, hd]
            # Waves of 3 chunk-groups, e-interleaved so the PE has work as
            # each (at_e, wv_e) DMA pair lands instead of waiting for all 8.
            for wave in ([0, 1, 2], [3, 4, 5], [6, 7]):
                pss = {mch: pps.tile([128, T], F32, tag="pp",
                                     name=f"pp{mch}") for mch in wave}
                for e in range(NC):
                    for mch in wave:
                        for ic in range(2):
                            nc.tensor.matmul(
                                pss[mch][:, bass.ts(ic, 512)],
                                at_sb[:, e, bass.ts(mch, 128)],
                                wv_sb[:, e, bass.ts(ic, 512)],
                                start=(e == 0), stop=(e == NC - 1))
                for mch in wave:
                    with nc.allow_low_precision(reason="V fp16 feeds fp16 matmul"):
                        nc.vector.tensor_copy(
                            v[:, mch, :].rearrange("p (h x) -> p h x", x=D + 1)[:, :, 0:D],
                            pss[mch][:].rearrange("p (h d) -> p h d", d=D))

            # Q^T/K^T chunk 0 (K first: wk arrives before wq)
            for (w_sb, act_sb, out_sb) in ((wk_sb, at_sb, kt), (wq_sb, bt_sb, qt)):
                ps = pps.tile([128, T], F32, tag="pp")
                for e in range(NC):
                    for ic in range(2):
                        nc.tensor.matmul(
                            ps[:, bass.ts(ic, 512)],
                            w_sb[:, e, bass.ts(0, 128)],
                            act_sb[:, e, bass.ts(ic, 512)],
                            start=(e == 0), stop=(e == NC - 1))
                with nc.allow_low_precision(reason="QK fp16 feeds fp16 matmul"):
                    nc.vector.tensor_copy(out_sb[:, 0, :], ps[:])

        # ---------------- flat slot loop: attention over 8 head pairs ----------
        with tc.tile_pool(name="sps", bufs=2, space="PSUM") as sps_pool, \
             tc.tile_pool(name="ops", bufs=1, space="PSUM") as ops, \
             tc.tile_pool(name="pjp", bufs=1, space="PSUM") as pjp, \
             tc.tile_pool(name="ptp", bufs=8) as ptp, \
             tc.tile_pool(name="oup", bufs=2) as oup, \
             tc.tile_pool(name="nrm", bufs=2) as nrm:

            SLOTS = [(m, ic, jc) for m in range(NC)
                     for ic in range(2) for jc in range(NC)]

            def issue_s(m, ic, jc):
                sps = sps_pool.tile([128, 1024], F32, tag="s", name="sps")
                nc.tensor.matmul(
                    sps[:, 0:512],
                    kt[0:64, m, bass.ts(jc, 128)],
                    qt[0:64, m, bass.ts(ic, 512)],
                    start=True, stop=True)
                nc.tensor.matmul(
                    sps[:, 512:1024],
                    kt[64:128, m, bass.ts(jc, 128)],
                    qt[64:128, m, bass.ts(ic, 512)],
                    start=True, stop=True, tile_position=(64, 0))
                return sps

            def issue_o(po):
                pt_prev, jc, psA, psB, hA, hB = po
                st = dict(start=(jc == 0), stop=(jc == NC - 1))
                nc.tensor.matmul(psA[:, :],
                                 v[:, jc, hA * (D + 1):(hA + 1) * (D + 1)],
                                 pt_prev[:, 0:512], **st)
                nc.tensor.matmul(psB[:, :],
                                 v[:, jc, hB * (D + 1):(hB + 1) * (D + 1)],
                                 pt_prev[:, 512:1024], **st)

            def issue_norm_a(pn, lo=0, hi=T):
                mm, ouA, ouB, rs2 = pn
                rr2 = nrm.tile([1, 2, T], F32, tag="rr2", bufs=1, name="rr2")
                nc.vector.reciprocal_approx_fast(rr2[:, :, lo:hi], rs2[:, :, lo:hi])
                rrh2 = nrm.tile([1, 2, T], F16, tag="rrh2", bufs=1, name="rrh2")
                with nc.allow_low_precision(reason="recip feeds fp16 multiply"):
                    nc.vector.tensor_copy(rrh2[:, :, lo:hi], rr2[:, :, lo:hi])
                bcA = nrm.tile([64, T], F16, tag="bcA", bufs=1, name="bcA")
                bcB = nrm.tile([64, T], F16, tag="bcB", bufs=1, name="bcB")
                nc.gpsimd.partition_broadcast(bcA[:, lo:hi], rrh2[:, 0, lo:hi])
                nc.gpsimd.partition_broadcast(bcB[:, lo:hi], rrh2[:, 1, lo:hi])
                return (mm, ouA, ouB, bcA, bcB, lo, hi)

            def issue_norm_b(h):
                # Muls stay on DVE: gpsimd would swap ucode libraries between
                # partition_broadcast and tensor_tensor every pair (~5-8us).
                mm, ouA, ouB, bcA, bcB, lo, hi = h
                with nc.allow_low_precision(reason="O^T fp16 feeds fp16 out-proj"):
                    nc.vector.tensor_mul(ot[0:64, mm, lo:hi], ouA[:, lo:hi],
                                         bcA[:, lo:hi])
                    nc.vector.tensor_mul(ot[64:128, mm, lo:hi], ouB[:, lo:hi],
                                         bcB[:, lo:hi])

            def issue_norm(pn, lo=0, hi=T):
                issue_norm_b(issue_norm_a(pn, lo, hi))

            def evac_half(psA, psB, ouA, ouB, rs2, ic):
                sl = bass.ts(ic, 512)
                with nc.allow_low_precision(reason="O' fp16 feeds fp16 multiply"):
                    nc.vector.tensor_copy(ouA[:, sl], psA[0:D, :])
                    nc.vector.tensor_copy(ouB[:, sl], psB[0:D, :])
                nc.vector.tensor_copy(rs2[:, 0, sl], psA[D:D + 1, :])
                nc.vector.tensor_copy(rs2[:, 1, sl], psB[D:D + 1, :])

            pending_o = None
            held_o = None       # O of the post-switch slot, held one body
            pending_norm = None
            norm_h = None       # issue_norm_a handles awaiting issue_norm_b
            m7h0_h = None       # same, for the last pair's s8 half-0 norm
            defer_q1 = None     # (pj1_tile, ch) q-icc1 cast deferred past evac
            defer_k1 = None     # (pj1_tile, ch) k-icc1 cast deferred past evac
            ps_oA = ps_oB = ouA = ouB = rs2 = None
            prev = None  # (ps_oA, ps_oB, ouA, ouB, rs2, m) of previous pair
            pj = {0: None, 1: None}

            pending_s = issue_s(*SLOTS[0])
            for gs, (m, ic, jc) in enumerate(SLOTS):
                s = gs % 16
                hA, hB = 2 * m, 2 * m + 1
                sps = pending_s
                pt_t = ptp.tile([128, 1024], F16, tag="pt")
                nc.scalar.activation(pt_t[:], sps[:], EXP, scale=0.125)

                po_prev, pending_o = pending_o, None
                if s in (0, 8) and po_prev is not None:
                    # the flush of the half-pair MUST be emitted before the
                    # evac below — program order defines the dependency graph
                    issue_o(po_prev)
                    po_prev = None

                if s == 0:
                    if gs > 0:
                        pA, pB, oA, oB, r2, pm = prev
                        evac_half(pA, pB, oA, oB, r2, 1)
                        pending_norm = (pm, oA, oB, r2)
                        if defer_k1 is not None:
                            tile_k1, chk = defer_k1
                            with nc.allow_low_precision(reason="QK fp16"):
                                nc.vector.tensor_copy(
                                    kt[:, chk, bass.ts(1, 512)], tile_k1[:, :])
                            defer_k1 = None
                    ps_oA = ops.tile([D + 1, 512], F32, tag="oA")
                    ps_oB = ops.tile([D + 1, 512], F32, tag="oB")
                    ouA = oup.tile([D, T], F16, tag="ouA")
                    ouB = oup.tile([D, T], F16, tag="ouB")
                    rs2 = nrm.tile([1, 2, T], F32, tag="rs2", bufs=2)
                    prev = (ps_oA, ps_oB, ouA, ouB, rs2, m)
                elif s == 8:
                    evac_half(ps_oA, ps_oB, ouA, ouB, rs2, 0)
                    if defer_q1 is not None:
                        tile_q1, chq = defer_q1
                        with nc.allow_low_precision(reason="QK fp16"):
                            nc.vector.tensor_copy(
                                qt[:, chq, bass.ts(1, 512)], tile_q1[:, :])
                        defer_q1 = None
                    if m == NC - 1:
                        m7h0_h = issue_norm_a((m, ouA, ouB, rs2), 0, 512)
                    ps_oA = ops.tile([D + 1, 512], F32, tag="oA")
                    ps_oB = ops.tile([D + 1, 512], F32, tag="oB")
                    prev = (ps_oA, ps_oB, ouA, ouB, rs2, m)
                elif s == 10 and m7h0_h is not None:
                    issue_norm_b(m7h0_h)
                    m7h0_h = None
                elif s == (2 if m == NC - 1 else 9) and pending_norm is not None:
                    # lagged norm of the previous pair, split into recip+bc
                    # here and the muls 4 slots later, so the k-icc0 cast in
                    # between isn't buried behind a 5us DVE burst; on the
                    # last pair it runs early (no proj casts there) so ot[6]
                    # is ready before the Z stage starts
                    norm_h = issue_norm_a(pending_norm)
                    pending_norm = None
                elif s == (4 if m == NC - 1 else 13) and norm_h is not None:
                    issue_norm_b(norm_h)
                    norm_h = None

                pending_o = (pt_t, jc, ps_oA, ps_oB, hA, hB)

                # interleaved Q^T/K^T projections for chunk m+1
                entry = _SCHED.get((m, s))
                if entry is not None:
                    ch = m + 1
                    for op in entry:
                        which, icc = op[1], op[2]
                        w_p, a_p, o_p = ((wq_sb, bt_sb, qt) if which == 'q'
                                         else (wk_sb, at_sb, kt))
                        if op[0] == 'mm':
                            e = op[3]
                            if pj[icc] is None:
                                pj[icc] = pjp.tile([128, 512], F32,
                                                   tag=f"pj{icc}",
                                                   name=f"pj{icc}")
                            nc.tensor.matmul(
                                pj[icc][:, :],
                                w_p[:, e, bass.ts(ch, 128)],
                                a_p[:, e, bass.ts(icc, 512)],
                                start=(e == 0), stop=(e == NC - 1))
                        else:
                            with nc.allow_low_precision(reason="QK fp16"):
                                nc.vector.tensor_copy(
                                    o_p[:, ch, bass.ts(icc, 512)],
                                    pj[icc][:, :])
                            pj[icc] = None
                    if s == 7:          # q-icc1 done; cast after the s8 evac
                        defer_q1 = (pj[1], ch)
                        pj[1] = None
                    elif s == 15:       # k-icc1 done; cast after next s0 evac
                        defer_k1 = (pj[1], ch)
                        pj[1] = None

                # S next-slot after proj (ACT(s-1) has freed its sps buffer
                # by the time the in-order PE gets here), then the lagged O
                # at the very tail: around the s0/s8 psum switches it waits
                # on the DVE evacuation, and at the tail it blocks nothing —
                # the proj/S matmuls above fill the round-trip bubble.
                pending_s = (issue_s(*SLOTS[gs + 1])
                             if gs + 1 < len(SLOTS) else None)
                if po_prev is not None:
                    issue_o(po_prev)  # at s==0 this flushed prev pair (stop)

            # drain: flush last O slot, evacuate half 1, final norm half 2
            issue_o(pending_o)
            pending_o = None
            evac_half(ps_oA, ps_oB, ouA, ouB, rs2, 1)
            issue_norm((NC - 1, ouA, ouB, rs2), 512, T)

        # ---------------- Z: out-projection (fp16 out) ----------------
        # cc pairs, ic-outer: the ic=0 columns only need ot halves normed at
        # the last pair's s8, so they stream while the final (ic=1) norm of
        # pair 7 is still in flight; only the first pair's ic=1 group waits.
        with tc.tile_pool(name="zps", bufs=2, space="PSUM") as zps, \
             tc.tile_pool(name="zsb", bufs=2) as zsbp:
            for cp in range(0, NC, 2):
                psz = {cc: zps.tile([128, T], F32, tag="z", name=f"z{cc}")
                       for cc in (cp, cp + 1)}
                for ic in range(2):
                    for cc in (cp, cp + 1):
                        for mm in range(NC):
                            nc.tensor.matmul(
                                psz[cc][:, bass.ts(ic, 512)],
                                wo_sb[:, mm, bass.ts(cc, 128)],
                                ot[:, mm, bass.ts(ic, 512)],
                                start=(mm == 0), stop=(mm == NC - 1))
                for cc in (cp, cp + 1):
                    zsb = zsbp.tile([128, T], F16, tag="zsb", name=f"zsb{cc}")
                    with nc.allow_low_precision(reason="fp16 output"):
                        nc.vector.tensor_copy(zsb[:], psz[cc][:])
                    nc.sync.dma_start(z_t[cc * 128:(cc + 1) * 128, :], zsb[:])
    nc.compile()
    return nc


def _group_w(wqkv, k):
    """Rows of Wqkv (3E, E) for q/k/v (k=0/1/2), grouped head-major.

    Row index layout: r = di*(3H) + k*H + h  ->  grouped[h*D+di, :].
    """
    w = np.asarray(wqkv, dtype=np.float32).reshape(D, 3, H, E)[:, k]   # [di, h, e]
    return np.ascontiguousarray(w.transpose(1, 0, 2).reshape(E, E))    # [h*D+di, e]


def kernel(x, y, Wqkv1, Wqkv2, Wout1, Wout2):
    x = np.asarray(x, dtype=np.float32)
    y = np.asarray(y, dtype=np.float32)

    if "nc" not in _NC_CACHE:
        _NC_CACHE["nc"] = _build()
    nc = _NC_CACHE["nc"]

    wq1_t = np.ascontiguousarray(_group_w(Wqkv1, 0).T)
    wk1_t = np.ascontiguousarray(_group_w(Wqkv1, 1).T)
    wv1_t = np.ascontiguousarray(_group_w(Wqkv1, 2).T)
    wq2_t = np.ascontiguousarray(_group_w(Wqkv2, 0).T)
    wk2_t = np.ascontiguousarray(_group_w(Wqkv2, 1).T)
    wv2_t = np.ascontiguousarray(_group_w(Wqkv2, 2).T)
    wout1_t = np.ascontiguousarray(np.asarray(Wout1, dtype=np.float32).T)
    wout2_t = np.ascontiguousarray(np.asarray(Wout2, dtype=np.float32).T)

    in_maps = []
    for c in range(N_CORES):
        s, b = divmod(c, B)
        if s == 0:
            # stream-1 output: K,V from x via Wqkv1; Q from y via Wqkv2
            a_t, b_t = x[b].T, y[b].T
            wq, wk, wv, wo = wq2_t, wk1_t, wv1_t, wout1_t
        else:
            a_t, b_t = y[b].T, x[b].T
            wq, wk, wv, wo = wq1_t, wk2_t, wv2_t, wout2_t
        in_maps.append({
            "a_t": np.ascontiguousarray(a_t).astype(np.float16),
            "b_t": np.ascontiguousarray(b_t).astype(np.float16),
            "wq_t": wq.astype(np.float16), "wk_t": wk.astype(np.float16),
            "wv_t": wv.astype(np.float16), "wout_t": wo.astype(np.float16),
        })

    trace = os.environ.get("BASS_KERNEL_TRACE", "0") == "1"
    if trace:
        try:
            from antenv.axon_hooks import get_axon_ntff_profile_hook  # noqa: F401
        except ImportError:
            trace = False
    ncores = int(os.environ.get("KCORES", str(N_CORES)))
    r = bass_utils.run_bass_kernel_spmd(nc, in_maps[:ncores], core_ids=list(range(ncores)),
                                        trace=trace)
    LAST_RESULTS["exec_time_ns"] = r.exec_time_ns
    LAST_RESULTS["profile_json"] = r.profile_json

    out1 = np.stack([r.results[b]["z_t"].T.astype(np.float32) for b in range(B)])
    out2 = np.stack([r.results[B + b]["z_t"].T.astype(np.float32) for b in range(B)])
    return out1, out2
